# revision 56
# baseline (speedup 1.0000x reference)
"""Trainium2 Bass kernel for an 8-batch AttentionBlock (GroupNorm + single-head
self-attention over 64x64 spatial + residual), data-parallel over batch on 8
NeuronCores (one batch element per core).

Per-core math (x: [512, 4096]):
  h   = groupnorm(x) * gamma + beta       (32 groups of 16 ch; h stored fp8)
  u   = (Wk^T Wq) h                       (folded q/k: one fp8 DoubleRow proj)
  s_j = (Wk^T bq).h_j / sqrt(C)           (per-j softmax bias; bk etc. cancel)
  vT  = (Wv h)^T                          (fp8, layout [j, c], direct - no
                                           transposes anywhere in the kernel)
  St  = h^T u          [j, i] blocks, fp8 DoubleRow matmuls into PSUM f32
  Pt  = exp(St*scale + s_j)  fp8 SBUF (ScalarE runs Exp only - no LUT swaps)
  dacc= sum_jb Pt      f32 SBUF [128, i]  (VectorE + GpSimd split accumulation)
  denb= ones128^T dacc PSUM (all-ones matmul = column-sum broadcast to all
                             partitions; ones=16 compensates the 16x fp8
                             weight prescale on the v path)
  attn= O * recip(denb)  bf16, O = vT^T Pt accumulated in PSUM (fp8 DoubleRow)
  out = xb + Wp attn   where xb = x + (Wp bv + bp) is host-folded f32, so the
                       residual path is exact
"""

import sys

if "/opt/trn_rl_repo" not in sys.path:
    sys.path.insert(0, "/opt/trn_rl_repo")

import math

import ml_dtypes
import numpy as np

C = 512
N = 4096
P = 128
CT = C // P      # 4 channel tiles
FB = 512         # free-dim block (i)
NB = N // FB     # 8 i-blocks
JB = N // P      # 32 j-blocks
GS = 16          # channels per group
EPS = 1e-5
PIPE = 5         # jb-pair delay between St/exp emission and den/O consumption

_CACHE = {}


def _build():
    import concourse.tile as tile
    from concourse import bacc, mybir

    f32 = mybir.dt.float32
    bf16 = mybir.dt.bfloat16
    f8 = mybir.dt.float8e4
    AF = mybir.ActivationFunctionType
    DR = mybir.MatmulPerfMode.DoubleRow

    nc = bacc.Bacc("TRN2", target_bir_lowering=False, debug=False, num_devices=8)

    # bf16 copy of x for the groupnorm/stats path (h is fp8 downstream, so
    # bf16 stats are plenty); the exact f32 x only enters via xb (residual).
    xh_d = nc.dram_tensor("xh", [C, N], bf16, kind="ExternalInput").ap()
    # weights arrive pre-tiled as [P, CT, C] so the load is one contiguous DMA.
    # wuT is the folded score matrix (Wk^T Wq, scaled): softmax(q.k) ==
    # softmax(h.(M h) + s[j]) where s[j] = (Wk^T bq).h_j -- bk and the
    # i-only bias terms cancel inside the softmax.
    wu_d = nc.dram_tensor("wuT", [P, CT, C], f8, kind="ExternalInput").ap()
    wv_d = nc.dram_tensor("wvT", [P, CT, C], f8, kind="ExternalInput").ap()
    wp_d = nc.dram_tensor("wpT", [P, CT, C], bf16, kind="ExternalInput").ap()
    ws_d = nc.dram_tensor("ws", [P, CT], f8, kind="ExternalInput").ap()
    # xb = x + (Wp bv + bp) per channel, pre-added on host: the entire
    # residual-plus-output-bias term, so the epilogue is one tensor_add.
    xb_d = nc.dram_tensor("xb", [C, N], f32, kind="ExternalInput").ap()
    gamma_d = nc.dram_tensor("gamma", [P, CT], f32, kind="ExternalInput").ap()
    beta_d = nc.dram_tensor("beta", [P, CT], f32, kind="ExternalInput").ap()
    g16_d = nc.dram_tensor("g16", [P, P // GS], f32, kind="ExternalInput").ap()
    gt_d = nc.dram_tensor("gt", [P // GS, P], f32, kind="ExternalInput").ap()
    out_d = nc.dram_tensor("out", [C, N], f32, kind="ExternalOutput").ap()

    with tile.TileContext(nc) as tc:
        from contextlib import ExitStack

        with ExitStack() as ctx:
            consts = ctx.enter_context(tc.tile_pool(name="consts", bufs=1))
            big = ctx.enter_context(tc.tile_pool(name="big", bufs=1))
            xpool = ctx.enter_context(tc.tile_pool(name="p1", bufs=CT))

            # x feeds the groupnorm critical path - issue its DMAs before
            # anything else lands on the sync queue (descriptor issue is
            # serial, ~0.6us each)
            x_tiles = []
            for ct in range(CT):
                x_t = xpool.tile([P, N], bf16, name="xt")
                nchunk = 4 if ct == 0 else 2
                for hh in range(nchunk):
                    sl = slice(hh * (N // nchunk), (hh + 1) * (N // nchunk))
                    eng = nc.sync if hh % 2 == 0 else nc.scalar
                    eng.dma_start(x_t[:, sl], xh_d[ct * P:(ct + 1) * P, sl])
                x_tiles.append(x_t)

            def load_w(dram, nm, dt):  # noqa: E306
                t = consts.tile([P, CT, C], dt, name=nm)
                nc.sync.dma_start(t[:], dram)
                return t

            wu_sb = load_w(wu_d, "wu_sb", f8)
            wv_sb = load_w(wv_d, "wv_sb", f8)
            wp_sb = load_w(wp_d, "wp_sb", bf16)

            def load_small(dram, shape, nm, dt=f32):
                # gpsimd queue: don't let these tiny loads (needed early by
                # the groupnorm chain) queue behind the big weight DMAs
                t = consts.tile(shape, dt, name=nm)
                nc.gpsimd.dma_start(t[:], dram)
                return t

            ws_sb = load_small(ws_d, [P, CT], "ws_sb", f8)
            gamma_sb = load_small(gamma_d, [P, CT], "gamma_sb")
            beta_sb = load_small(beta_d, [P, CT], "beta_sb")
            g16_sb = load_small(g16_d, [P, P // GS], "g16_sb")
            gt_sb = load_small(gt_d, [P // GS, P], "gt_sb")

            # fp8 weights are pre-scaled by 16x on the host (keeps them out of
            # e4m3's subnormal range); v therefore comes out 16x hot, which
            # cancels in attn = O/den when den is also summed with weight 16.
            ones128 = consts.tile([P, P], bf16, name="ones128")
            nc.vector.memset(ones128[:], 16.0)
            eps_sb = consts.tile([P // GS, 1], f32, name="eps_sb")
            nc.vector.memset(eps_sb[:], EPS)

            u_sb = big.tile([P, CT, N], f8, name="u")
            vt_sb = big.tile([P, JB, C], f8, name="vt")
            h_sb = big.tile([P, CT, N], f8, name="h")
            # per-j additive softmax bias s[j] (see wuT comment), f32
            st_bias = big.tile([P, JB], f32, name="st_bias")

            # shared matmul psum pool (projections phase + St blocks)
            sps = ctx.enter_context(tc.tile_pool(name="sps", bufs=3, space="PSUM"))

            # ---------------- phase 1+2: groupnorm -> h -> u/vT/s ----------
            if True:
                with tc.tile_pool(name="p1s", bufs=2) as p1s, \
                     tc.tile_pool(name="gnps", bufs=1, space="PSUM") as gnps:
                    # dummy matmuls warm the PE HAM clock-gate (~3.4us of
                    # activity -> 2.4GHz) while DVE runs the stats chain;
                    # PE would otherwise sit idle and start projections cold
                    warm = gnps.tile([P, P], f32, name="warm")
                    for _ in range(64):
                        nc.tensor.matmul(warm[:], lhsT=ones128[:],
                                         rhs=ones128[:], start=True, stop=True)
                    for ct in range(CT):
                        x_t = x_tiles[ct]
                        stats = p1s.tile([P, 8, 6], f32, name="stats")
                        for sg in range(8):
                            nc.vector.bn_stats(
                                stats[:, sg, :], x_t[:, sg * 512:(sg + 1) * 512])
                        mv = p1s.tile([P, 2], f32, name="mv")
                        nc.vector.bn_aggr(mv[:], stats[:])
                        # ms = [mean, E[x^2]] per channel; gpsimd keeps these
                        # tiny ops out of the serial DVE bn_stats chain
                        ms = p1s.tile([P, 2], f32, name="ms")
                        nc.gpsimd.tensor_copy(ms[:, 0:1], mv[:, 0:1])
                        nc.gpsimd.tensor_mul(ms[:, 1:2], mv[:, 0:1], mv[:, 0:1])
                        nc.gpsimd.tensor_add(ms[:, 1:2], ms[:, 1:2], mv[:, 1:2])
                        # group aggregate: [8, 2] = (gmean, gE[x^2])
                        gps = gnps.tile([P // GS, 2], f32, name="gps")
                        nc.tensor.matmul(gps[:], lhsT=g16_sb[:], rhs=ms[:],
                                         start=True, stop=True)
                        gsb = p1s.tile([P // GS, 2], f32, name="gsb")
                        nc.vector.tensor_copy(gsb[:], gps[:])
                        gm2 = p1s.tile([P // GS, 1], f32, name="gm2")
                        nc.gpsimd.tensor_mul(gm2[:], gsb[:, 0:1], gsb[:, 0:1])
                        nc.gpsimd.tensor_sub(gsb[:, 1:2], gsb[:, 1:2], gm2[:])
                        # gsb[:,1] = 1/sqrt(gvar + eps)
                        nc.scalar.activation(gsb[:, 1:2], gsb[:, 1:2], AF.Sqrt,
                                             bias=eps_sb[:], scale=1.0)
                        nc.vector.reciprocal_approx_fast(gsb[:, 1:2],
                                                         gsb[:, 1:2])
                        # broadcast group (mean, rstd) back to 128 channels
                        cps = gnps.tile([P, 2], f32, name="cps")
                        nc.tensor.matmul(cps[:], lhsT=gt_sb[:], rhs=gsb[:],
                                         start=True, stop=True)
                        scale_t = p1s.tile([P, 1], f32, name="scale")
                        nc.vector.tensor_mul(scale_t[:], cps[:, 1:2],
                                             gamma_sb[:, ct:ct + 1])
                        nbias_t = p1s.tile([P, 1], f32, name="nbias")
                        nc.vector.tensor_mul(nbias_t[:], cps[:, 0:1], scale_t[:])
                        nc.gpsimd.tensor_sub(nbias_t[:], beta_sb[:, ct:ct + 1],
                                             nbias_t[:])
                        for hh in range(2):
                            sl = slice(hh * (N // 2), (hh + 1) * (N // 2))
                            nc.vector.tensor_scalar(
                                out=h_sb[:, ct, sl], in0=x_t[:, sl],
                                scalar1=scale_t[:], scalar2=nbias_t[:],
                                op0=mybir.AluOpType.mult,
                                op1=mybir.AluOpType.add)

                # u i-block-major (St(ib=0) needs u[:, :, 0:FB]), then vT+s.
                for ib in range(NB):
                    for ct in range(CT):
                        qp = sps.tile([P, FB], f32, name="st")
                        for kt in range(0, CT, 2):
                            nc.tensor.matmul(
                                qp[:],
                                lhsT=wu_sb[:, kt:kt + 2, ct * P:(ct + 1) * P],
                                rhs=h_sb[:, kt:kt + 2, ib * FB:(ib + 1) * FB],
                                start=(kt == 0), stop=(kt == CT - 2),
                                perf_mode=DR)
                        nc.scalar.activation(
                            u_sb[:, ct, ib * FB:(ib + 1) * FB], qp[:],
                            AF.Identity, bias=0.0, scale=1.0)
                with tc.tile_pool(name="stps", bufs=2, space="PSUM") as stp:
                    for jb in range(JB):
                        vp = sps.tile([P, C], f32, name="st")
                        sp = stp.tile([P, 1], f32, name="sp")
                        for kt in range(0, CT, 2):
                            nc.tensor.matmul(
                                vp[:],
                                lhsT=h_sb[:, kt:kt + 2, jb * P:(jb + 1) * P],
                                rhs=wv_sb[:, kt:kt + 2, :],
                                start=(kt == 0), stop=(kt == CT - 2),
                                perf_mode=DR)
                            nc.tensor.matmul(
                                sp[:],
                                lhsT=h_sb[:, kt:kt + 2, jb * P:(jb + 1) * P],
                                rhs=ws_sb[:, kt:kt + 2, None],
                                start=(kt == 0), stop=(kt == CT - 2),
                                perf_mode=DR)
                        nc.scalar.activation(vt_sb[:, jb, :], vp[:], AF.Identity,
                                             bias=0.0, scale=1.0)
                        # ws carries a 128x host scale (fp8 subnormal guard)
                        nc.scalar.activation(st_bias[:, jb:jb + 1], sp[:],
                                             AF.Identity, bias=0.0,
                                             scale=1.0 / 128.0)

            # ---------------- phase 3+4: attention + output projection ------
            with tc.tile_pool(name="attnpool", bufs=1) as apool, \
                 tc.tile_pool(name="ptpool", bufs=12) as ptp, \
                 tc.tile_pool(name="ops", bufs=1, space="PSUM") as ops, \
                 tc.tile_pool(name="dps", bufs=1, space="PSUM") as dps, \
                 tc.tile_pool(name="dpool", bufs=2) as dpool, \
                 tc.tile_pool(name="mpool", bufs=2) as mp, \
                 tc.tile_pool(name="xrpool", bufs=5) as xrp, \
                 tc.tile_pool(name="outpool", bufs=3) as outp:
                attn_sb = apool.tile([P, CT, N], bf16, name="attn")

                def final_proj(ib):
                    xrs = []
                    for ct in range(CT):
                        xr = xrp.tile([P, FB], f32, name="xr")
                        nc.sync.dma_start(
                            xr[:], xb_d[ct * P:(ct + 1) * P, ib * FB:(ib + 1) * FB])
                        xrs.append(xr)
                    for ct in range(CT):
                        yp = dps.tile([P, FB], f32, name="scr")
                        for kt in range(CT):
                            nc.tensor.matmul(
                                yp[:],
                                lhsT=wp_sb[:, kt, ct * P:(ct + 1) * P],
                                rhs=attn_sb[:, kt, ib * FB:(ib + 1) * FB],
                                start=(kt == 0), stop=(kt == CT - 1))
                        ot = outp.tile([P, FB], f32, name="ot")
                        nc.vector.tensor_add(ot[:], yp[:], xrs[ct][:])
                        nc.sync.dma_start(
                            out_d[ct * P:(ct + 1) * P, ib * FB:(ib + 1) * FB],
                            ot[:])

                JP = JB // 2  # j-block pairs (DoubleRow packs 2 k-subtiles)
                for ib in range(NB):
                    o_tiles = [ops.tile([P, FB], f32, name=f"o{cs}")
                               for cs in range(CT)]
                    # two independent denominator accumulators halve the
                    # serial DVE chain and tolerate scheduling jitter
                    # bf16 accumulators: rounding error averages out 1/sqrt(128)
                    # in the column-sum matmul, and bf16 lhsT/rhs makes the den
                    # matmul single-pass (fp32 matmuls run LOW+HIGH twice)
                    dacc = [dpool.tile([P, FB], bf16, name=f"dacc{h}")
                            for h in range(2)]
                    pt_q = []

                    def consume(jp, pt):
                        # accumulator 0 on VectorE, accumulator 1 on the
                        # otherwise-idle GpSimd so neither serial chain gates
                        # pt-tile reuse
                        for h, eng in ((0, nc.vector), (1, nc.gpsimd)):
                            if jp == 0:
                                eng.tensor_copy(dacc[h][:], pt[:, h, :])
                            else:
                                eng.tensor_add(dacc[h][:], dacc[h][:],
                                               pt[:, h, :])
                        for cs in range(CT):
                            nc.tensor.matmul(
                                o_tiles[cs][:],
                                lhsT=vt_sb[:, 2 * jp:2 * jp + 2,
                                           cs * P:(cs + 1) * P],
                                rhs=pt[:],
                                start=(jp == 0), stop=(jp == JP - 1),
                                perf_mode=DR)

                    for jp in range(JP):
                        pt = ptp.tile([P, 2, FB], f8, name="pt")
                        for h in range(2):
                            jb = 2 * jp + h
                            st = sps.tile([P, FB], f32, name="st")
                            for kt in range(0, CT, 2):
                                nc.tensor.matmul(
                                    st[:],
                                    lhsT=h_sb[:, kt:kt + 2,
                                              jb * P:(jb + 1) * P],
                                    rhs=u_sb[:, kt:kt + 2,
                                             ib * FB:(ib + 1) * FB],
                                    start=(kt == 0), stop=(kt == CT - 2),
                                    perf_mode=DR)
                            # wuT carries a 32x host scale; undo it plus the
                            # 1/sqrt(C) attention scale inside the exp, and add
                            # the per-j softmax bias s[j]
                            nc.scalar.activation(pt[:, h, :], st[:], AF.Exp,
                                                 bias=st_bias[:, jb:jb + 1],
                                                 scale=1.0 / (32.0 * math.sqrt(C)))
                        pt_q.append((jp, pt))
                        if jp == PIPE and ib > 0:
                            # overlap previous block's output projection with
                            # this block's score matmuls
                            final_proj(ib - 1)
                        if jp >= PIPE:
                            consume(*pt_q.pop(0))
                    while pt_q:
                        consume(*pt_q.pop(0))

                    # all-ones matmul: every psum partition gets sum_j dacc[j,:]
                    denb = dps.tile([P, FB], f32, name="scr")
                    nc.tensor.matmul(denb[:], lhsT=ones128[:], rhs=dacc[0][:],
                                     start=True, stop=False)
                    nc.tensor.matmul(denb[:], lhsT=ones128[:], rhs=dacc[1][:],
                                     start=False, stop=True)
                    rdb = mp.tile([P, FB], f32, name="rdb")
                    nc.vector.reciprocal_approx_fast(rdb[:], denb[:])
                    for cs in range(CT):
                        nc.vector.tensor_mul(
                            attn_sb[:, cs, ib * FB:(ib + 1) * FB],
                            o_tiles[cs][:], rdb[:])
                final_proj(NB - 1)

    nc.compile()
    return nc


def _host_inputs(x, gamma, beta, Wq, bq, Wk, bk, Wv, bv, Wp, bp):
    bf16 = ml_dtypes.bfloat16
    f32 = np.float32
    B = x.shape[0]
    s = 1.0 / math.sqrt(C)
    xs = np.asarray(x, f32).reshape(B, C, N)

    def fold(v):
        return np.asarray(v, f32).reshape(CT, P).T.copy()

    f8 = ml_dtypes.float8_e4m3fn

    def wtile(w, scale, dt):
        # [Cout, Cin] -> transposed [Cin, Cout] -> tiled [P, CT, Cout]
        wT = np.asarray(w, f32).T * scale
        return np.ascontiguousarray(
            wT.reshape(CT, P, C).transpose(1, 0, 2)).astype(dt)

    # folded score matrix: softmax_j(q_i.k_j/sqrt(C)) with q=Wq h+bq,
    # k=Wk h+bk equals softmax_j(h_j.(M h_i)/sqrt(C) + s_j) with
    # M = Wk^T Wq and s = (Wk^T bq).h_j/sqrt(C); bk and i-only terms cancel.
    M = np.asarray(Wk, f32).T @ np.asarray(Wq, f32)
    wsv = (np.asarray(Wk, f32).T @ np.asarray(bq, f32)) / math.sqrt(C)
    common = {
        "wuT": wtile(M, 32.0, f8),
        "wvT": wtile(Wv, 16.0, f8),
        "wpT": wtile(Wp, 1.0, bf16),
        "ws": (wsv * 128.0).reshape(CT, P).T.copy().astype(f8),
        "gamma": fold(gamma),
        "beta": fold(beta),
    }
    bias_out = (np.asarray(Wp, f32) @ np.asarray(bv, f32)
                + np.asarray(bp, f32)).astype(f32)
    xbs = xs + bias_out[None, :, None]
    g16 = np.zeros((P, P // GS), f32)
    g16[np.arange(P), np.arange(P) // GS] = 1.0 / GS
    gt = np.zeros((P // GS, P), f32)
    gt[np.arange(P) // GS, np.arange(P)] = 1.0
    common["g16"] = g16
    common["gt"] = gt
    return [dict(common, xh=np.ascontiguousarray(xs[b]).astype(bf16),
                 xb=np.ascontiguousarray(xbs[b])) for b in range(B)]


def kernel(x, gamma, beta, Wq, bq, Wk, bk, Wv, bv, Wp, bp, _trace=False):
    from concourse.bass_utils import run_bass_kernel_spmd

    if "nc" not in _CACHE:
        _CACHE["nc"] = _build()
    nc = _CACHE["nc"]
    in_maps = _host_inputs(x, gamma, beta, Wq, bq, Wk, bk, Wv, bv, Wp, bp)
    B = len(in_maps)
    res = run_bass_kernel_spmd(nc, in_maps, core_ids=list(range(B)),
                               trace=_trace)
    out = np.stack([res.results[b]["out"] for b in range(B)])
    out = out.reshape(x.shape).astype(np.float32)
    if _trace:
        _CACHE["last_results"] = res
    return out
